# revision 10
# baseline (speedup 1.0000x reference)
"""Trainium2 Bass kernel for nn_Attention_28802050687686.

GQA sliding-window attention, T=4096, D=2048, 8 Q heads / 4 KV heads,
head_dim 256, window 1024, tanh soft-cap 50, RMSNorm+RoPE on Q/K, RMSNorm on V.

Sharding: sequence-parallel over 8 NeuronCores. Core c owns queries
[512c, 512c+512). Each core computes K/V for its OWN 512 rows only, then an
AllGather (via DRAM) distributes K/V; each core DMAs just its 1536-position
sliding window back into SBUF using partition-id-indexed dynamic offsets
(wrapped mod 8 — out-of-range chunks land in fully-masked positions).
"""
import sys

sys.path.insert(0, "/opt/trn_rl_repo")

import numpy as np
import ml_dtypes

import concourse.bass as bass
import concourse.tile as tile
from concourse import bacc, mybir
from concourse.bass_utils import run_bass_kernel_spmd

F32 = mybir.dt.float32
BF16 = mybir.dt.bfloat16
AF = mybir.ActivationFunctionType
OP = mybir.AluOpType

# problem constants
T, D, NH, KV, H, HH = 4096, 2048, 8, 4, 256, 128
N_CORES = 8
TC = 512          # queries / own kv rows per core
SW = 1536         # kv window per core
NST = SW // 128   # 12 s-tiles in window
NOT = TC // 128   # 4 own s-tiles
NDT = D // 128    # 16 d-tiles
NTT = TC // 128   # 4 t-tiles
WINDOW = 1024
SOFT_CAP = 50.0
EPS = 1e-6
ROPE_BASE = 10000.0

KCOLS = NH * TC            # 4096 cols of K in the kv-local pack (8 htiles x 512)
VCOLS = NOT * KV * 256     # 4096 cols of V pack
KVCOLS = KCOLS + VCOLS     # 8192


def build_program():
    nc = bacc.Bacc("TRN2", target_bir_lowering=False, debug=False)

    xq = nc.dram_tensor("xq", [D, TC], BF16, kind="ExternalInput").ap()
    qw = nc.dram_tensor("qw", [D, NH * H], BF16, kind="ExternalInput").ap()
    kwk = nc.dram_tensor("kwk", [D, KV * H], BF16, kind="ExternalInput").ap()
    kwv = nc.dram_tensor("kwv", [D, KV * H], BF16, kind="ExternalInput").ap()
    ow = nc.dram_tensor("ow", [NH * H, D], BF16, kind="ExternalInput").ap()
    cosq = nc.dram_tensor("cosq", [HH, TC], F32, kind="ExternalInput").ap()
    sinq = nc.dram_tensor("sinq", [HH, TC], F32, kind="ExternalInput").ap()
    maskT = nc.dram_tensor("maskT", [NST, 128, TC], F32, kind="ExternalInput").ap()
    inv2q = nc.dram_tensor("inv2q", [HH, 2], BF16, kind="ExternalInput").ap()
    inv2k = nc.dram_tensor("inv2k", [HH, 2], BF16, kind="ExternalInput").ap()
    inv2v = nc.dram_tensor("inv2v", [1, KV * H], F32, kind="ExternalInput").ap()
    out = nc.dram_tensor("out", [TC, D], F32, kind="ExternalOutput").ap()

    kvlocal = nc.dram_tensor("kvlocal", [128, KVCOLS], BF16).ap()
    kvgath = nc.dram_tensor("kvgath", [N_CORES * 128, KVCOLS], BF16,
                            addr_space="Shared").ap()

    with tile.TileContext(nc) as tc:
        with tc.tile_pool(name="persist", bufs=1) as persist, \
             tc.tile_pool(name="work", bufs=2) as work, \
             tc.tile_pool(name="owp", bufs=2) as owp:
            kT_sb = persist.tile([128, KV * 2, SW], BF16)     # 24 KB/p
            V_sb = persist.tile([128, NST, KV, 256], BF16)    # 24 KB/p
            qT_sb = persist.tile([128, NH * 2, TC], BF16)     # 16 KB/p
            encT_sb = persist.tile([128, NH * 2, TC], BF16)   # 16 KB/p
            xq_ch = []
            for ch in range(4):
                xc = persist.tile([128, NDT // 4, TC], BF16, name=f"xq{ch}")
                nc.sync.dma_start(
                    xc[:], xq[ch * (D // 4):(ch + 1) * (D // 4), :].rearrange(
                        "(dt p) s -> p dt s", p=128))
                xq_ch.append(xc)

            def xq_sb(dt):
                return xq_ch[dt // 4][:, dt % 4, :]
            cosq_sb = persist.tile([HH, TC], F32)
            nc.sync.dma_start(cosq_sb[:], cosq[:])
            sinq_sb = persist.tile([HH, TC], F32)
            nc.sync.dma_start(sinq_sb[:], sinq[:])
            inv2q_sb = persist.tile([HH, 2], BF16)
            nc.sync.dma_start(inv2q_sb[:], inv2q[:])
            inv2k_sb = persist.tile([HH, 2], BF16)
            nc.sync.dma_start(inv2k_sb[:], inv2k[:])
            inv2v_sb = persist.tile([128, KV * H], F32)       # 4 KB/p
            nc.sync.dma_start(inv2v_sb[:], inv2v.to_broadcast([128, KV * H]))
            epsq1 = persist.tile([1, 1], F32)
            nc.vector.memset(epsq1[:], float(H) * EPS)
            epsk1 = persist.tile([1, 1], F32)
            nc.vector.memset(epsk1[:], EPS)
            eps128 = persist.tile([128, 1], F32)
            nc.vector.memset(eps128[:], EPS)
            ones_f = persist.tile([1, 128], F32)
            nc.vector.memset(ones_f[:], 1.0)
            ones_b = persist.tile([128, 1], BF16)
            nc.vector.memset(ones_b[:], 1.0)

            def rope_norm_fold(ps_pair, inv2_sb, eps_t, dst0, dst1, bcast):
                """RMSNorm (exact via inv2 weights) + RoPE on an h-pair PSUM
                [128, 2, TC]; writes bf16 to dst0/dst1 [128, TC]."""
                sq0 = work.tile([128, TC], BF16, tag="wsq", name="sq0")
                nc.scalar.activation(sq0[:], ps_pair[:, 0, :], AF.Square)
                sq1 = work.tile([128, TC], BF16, tag="wsq", name="sq1")
                nc.scalar.activation(sq1[:], ps_pair[:, 1, :], AF.Square)
                rps = ps12.tile([1, TC], F32, tag="rowps", name="rps")
                nc.tensor.matmul(rps[:], inv2_sb[:, 0:1], sq0[:],
                                 start=True, stop=False)
                nc.tensor.matmul(rps[:], inv2_sb[:, 1:2], sq1[:],
                                 start=False, stop=True)
                srow = work.tile([1, TC], F32, tag="srow", name="srow")
                nc.scalar.activation(srow[:], rps[:], AF.Sqrt, bias=eps_t[:])
                rrow = work.tile([1, TC], F32, tag="rrow", name="rrow")
                nc.vector.reciprocal(rrow[:], srow[:])
                if bcast == "gpsimd":
                    rb = work.tile([128, TC], F32, tag="rb", name="rb")
                    nc.gpsimd.partition_broadcast(rb[:], rrow[:])
                else:
                    rb = ps12.tile([128, TC], F32, tag="psv", name="rbps")
                    nc.tensor.matmul(rb[:], ones_f[:], rrow[:], start=True, stop=True)
                ta = work.tile([128, TC], F32, tag="wf", name="ta")
                nc.vector.tensor_tensor(ta[:], ps_pair[:, 0, :], cosq_sb[:], OP.mult)
                tb = work.tile([128, TC], F32, tag="wf", name="tb")
                nc.vector.tensor_tensor(tb[:], ps_pair[:, 1, :], sinq_sb[:], OP.mult)
                nc.vector.tensor_tensor(ta[:], ta[:], tb[:], OP.subtract)
                nc.vector.tensor_tensor(dst0, ta[:], rb[:], OP.mult)
                ta2 = work.tile([128, TC], F32, tag="wf", name="ta2")
                nc.vector.tensor_tensor(ta2[:], ps_pair[:, 1, :], cosq_sb[:], OP.mult)
                tb2 = work.tile([128, TC], F32, tag="wf", name="tb2")
                nc.vector.tensor_tensor(tb2[:], ps_pair[:, 0, :], sinq_sb[:], OP.mult)
                nc.vector.tensor_tensor(ta2[:], ta2[:], tb2[:], OP.add)
                nc.vector.tensor_tensor(dst1, ta2[:], rb[:], OP.mult)

            # ---------------- phase A: own-row K/V projections ----------------
            own0 = SW - TC  # own rows start at window col 1024
            with tc.tile_pool(name="wp", bufs=2) as wp, \
                 tc.tile_pool(name="ps12", bufs=2, space="PSUM") as ps12:
                pending = None
                for k in range(KV):
                    wk_sb = wp.tile([128, NDT, H], BF16, tag="wh", name="wk")
                    nc.sync.dma_start(
                        wk_sb[:],
                        kwk[:, k * H:(k + 1) * H].rearrange("(dt p) h -> p dt h", p=128))
                    psp = ps12.tile([128, 2, TC], F32, tag="pspair", name="pspK")
                    for hh in range(2):
                        for dt in range(NDT):
                            nc.tensor.matmul(
                                psp[:, hh, :],
                                wk_sb[:, dt, hh * 128:(hh + 1) * 128],
                                xq_sb(dt),
                                start=(dt == 0), stop=(dt == NDT - 1))
                    if pending is not None:
                        pp, pk = pending
                        rope_norm_fold(pp, inv2k_sb, epsk1,
                                       kT_sb[:, pk * 2 + 0, own0:SW],
                                       kT_sb[:, pk * 2 + 1, own0:SW], "gpsimd")
                    pending = (psp, k)
                pp, pk = pending
                rope_norm_fold(pp, inv2k_sb, epsk1,
                               kT_sb[:, pk * 2 + 0, own0:SW],
                               kT_sb[:, pk * 2 + 1, own0:SW], "gpsimd")

                def v_epilogue(psv, k, st):
                    sqv = work.tile([128, H], F32, tag="sqv", name="sqv")
                    nc.scalar.activation(sqv[:], psv[:], AF.Square)
                    sqw = work.tile([128, H], F32, tag="sqw", name="sqw")
                    nc.vector.tensor_tensor(
                        sqw[:], sqv[:], inv2v_sb[:, k * H:(k + 1) * H], OP.mult)
                    rv2 = work.tile([128, 1], F32, tag="rv2", name="rv2")
                    nc.vector.tensor_reduce(rv2[:], sqw[:],
                                            mybir.AxisListType.X, OP.add)
                    srv = work.tile([128, 1], F32, tag="srv", name="srv")
                    nc.scalar.activation(srv[:], rv2[:], AF.Sqrt, bias=eps128[:])
                    rv = work.tile([128, 1], F32, tag="rv", name="rv")
                    nc.vector.reciprocal(rv[:], srv[:])
                    nc.vector.tensor_scalar_mul(
                        V_sb[:, NST - NOT + st, k, :], psv[:], rv[:])

                pend_v = None
                for k in range(KV):
                    vw_sb = wp.tile([128, NDT, H], BF16, tag="wh", name="vw")
                    nc.sync.dma_start(
                        vw_sb[:],
                        kwv[:, k * H:(k + 1) * H].rearrange("(dt p) h -> p dt h", p=128))
                    for st in range(NOT):
                        psv = ps12.tile([128, H], F32, tag="psv", name="psv")
                        for dt in range(NDT):
                            nc.tensor.matmul(
                                psv[:],
                                xq_sb(dt)[:, st * 128:(st + 1) * 128],
                                vw_sb[:, dt, :],
                                start=(dt == 0), stop=(dt == NDT - 1))
                        if pend_v is not None:
                            v_epilogue(*pend_v)
                        pend_v = (psv, k, st)
                v_epilogue(*pend_v)

                # stage own K/V to DRAM, all-gather, pull window chunks back
                nc.sync.dma_start(
                    kvlocal[:, 0:KCOLS].rearrange("p (a b) -> p a b", a=NH),
                    kT_sb[:, :, own0:SW])
                nc.sync.dma_start(
                    kvlocal[:, KCOLS:KVCOLS].rearrange(
                        "p (a k c) -> p a k c", a=NOT, k=KV),
                    V_sb[:, NST - NOT:NST, :, :])
                nc.gpsimd.collective_compute(
                    "AllGather", OP.bypass,
                    replica_groups=[list(range(N_CORES))],
                    ins=[kvlocal[:]], outs=[kvgath[:]],
                )

                # ------------- phase B1: Q projections (overlap gather) -------
                pend_q = None
                for n in range(NH):
                    wq_sb = wp.tile([128, NDT, H], BF16, tag="wh", name="wq")
                    nc.sync.dma_start(
                        wq_sb[:],
                        qw[:, n * H:(n + 1) * H].rearrange("(dt p) h -> p dt h", p=128))
                    psp = ps12.tile([128, 2, TC], F32, tag="pspair", name="pspQ")
                    for hh in range(2):
                        for dt in range(NDT):
                            nc.tensor.matmul(
                                psp[:, hh, :],
                                wq_sb[:, dt, hh * 128:(hh + 1) * 128],
                                xq_sb(dt),
                                start=(dt == 0), stop=(dt == NDT - 1))
                    if pend_q is not None:
                        pp, pn = pend_q
                        rope_norm_fold(pp, inv2q_sb, epsq1,
                                       qT_sb[:, pn * 2 + 0, :],
                                       qT_sb[:, pn * 2 + 1, :], "pe")
                    pend_q = (psp, n)
                pp, pn = pend_q
                rope_norm_fold(pp, inv2q_sb, epsq1,
                               qT_sb[:, pn * 2 + 0, :], qT_sb[:, pn * 2 + 1, :],
                               "pe")

                # window chunks j=0,1 from the gather (j=2 == own rows, already
                # in place). chunk index wraps mod 8: for cores 0/1 the wrapped
                # chunks are garbage but land in fully-masked window positions.
                pid = nc.sync.partition_id()
                for j in range(2):
                    cj = ((pid + 6 + j) % N_CORES) * 128
                    src = kvgath[bass.ds(cj, 128), :]
                    nc.sync.dma_start(
                        kT_sb[:, :, j * TC:(j + 1) * TC],
                        src[:, 0:KCOLS].rearrange("p (a b) -> p a b", a=NH * 2))
                    nc.sync.dma_start(
                        V_sb[:, NOT * j:NOT * (j + 1), :, :],
                        src[:, KCOLS:KVCOLS].rearrange(
                            "p (a k c) -> p a k c", a=NOT, k=KV))

            # ---------------- phase B2: attention ----------------
            with tc.tile_pool(name="p3", bufs=1) as p3, \
                 tc.tile_pool(name="aw", bufs=4) as aw, \
                 tc.tile_pool(name="ps3", bufs=2, space="PSUM") as ps3, \
                 tc.tile_pool(name="psenc", bufs=2, space="PSUM") as psenc:
                maskT_sb = p3.tile([128, NST, TC], F32)       # 24 KB/p
                nc.sync.dma_start(maskT_sb[:], maskT.rearrange("j p t -> p j t"))

                for g in range(KV):
                    heads = (2 * g, 2 * g + 1)
                    encs = [psenc.tile([128, 2, TC], F32, tag="enc",
                                       name=f"enc{a}") for a in range(2)]
                    dens = [ps3.tile([1, TC], F32, tag="den", name=f"den{a}")
                            for a in range(2)]
                    def pv_step(pTs, st):
                        # enc.T[h, t] += V.T @ P.T ; den[t] += sum_s P.T
                        for a in range(2):
                            for hh in range(2):
                                nc.tensor.matmul(
                                    encs[a][:, hh, :],
                                    V_sb[:, st, g, hh * 128:(hh + 1) * 128],
                                    pTs[a][:],
                                    start=(st == 0), stop=(st == NST - 1))
                            nc.tensor.matmul(
                                dens[a][:], ones_b[:], pTs[a][:],
                                start=(st == 0), stop=(st == NST - 1))

                    pend_pv = None
                    for st in range(NST):
                        pTs = []
                        for a, n in enumerate(heads):
                            psL = ps3.tile([128, TC], F32, tag="psL", name="psL")
                            for hh in range(2):
                                nc.tensor.matmul(
                                    psL[:],
                                    kT_sb[:, g * 2 + hh, st * 128:(st + 1) * 128],
                                    qT_sb[:, n * 2 + hh, :],
                                    start=(hh == 0), stop=(hh == 1))
                            t1 = aw.tile([128, TC], F32, tag="t1", name="t1", bufs=3)
                            nc.scalar.activation(t1[:], psL[:], AF.Tanh,
                                                 scale=1.0 / SOFT_CAP)
                            nc.vector.tensor_tensor(t1[:], t1[:],
                                                    maskT_sb[:, st, :], OP.add)
                            pT = aw.tile([128, TC], BF16, tag="pT", name="pT")
                            nc.scalar.activation(pT[:], t1[:], AF.Exp,
                                                 scale=SOFT_CAP)
                            pTs.append(pT)
                        if pend_pv is not None:
                            pv_step(*pend_pv)
                        pend_pv = (pTs, st)
                    pv_step(*pend_pv)
                    for a, n in enumerate(heads):
                        drow = aw.tile([1, TC], F32, tag="drow", name="drow", bufs=2)
                        nc.vector.reciprocal(drow[:], dens[a][:])
                        rbden = aw.tile([128, TC], F32, tag="rbden", name="rbden", bufs=2)
                        nc.gpsimd.partition_broadcast(rbden[:], drow[:])
                        for hh in range(2):
                            nc.vector.tensor_tensor(
                                encT_sb[:, n * 2 + hh, :], encs[a][:, hh, :],
                                rbden[:], OP.mult)

            # ---------------- phase C: output projection ----------------
            with tc.tile_pool(name="outp", bufs=3) as outp, \
                 tc.tile_pool(name="ps4", bufs=4, space="PSUM") as ps4:
                for dc in range(4):
                    ow_sb = owp.tile([128, NH * 2, 512], BF16, tag="ow", name="ow_sb")
                    nc.sync.dma_start(
                        ow_sb[:],
                        ow[:, dc * 512:(dc + 1) * 512].rearrange(
                            "(nh p) d -> p nh d", p=128))
                    for tt in range(NTT):
                        psO = ps4.tile([128, 512], F32, tag="psO", name="psO")
                        for nh in range(NH * 2):
                            nc.tensor.matmul(
                                psO[:],
                                encT_sb[:, nh, tt * 128:(tt + 1) * 128],
                                ow_sb[:, nh, :],
                                start=(nh == 0), stop=(nh == NH * 2 - 1))
                        ob = outp.tile([128, 512], F32, tag="ob", name="ob")
                        nc.vector.tensor_copy(ob[:], psO[:])
                        nc.sync.dma_start(
                            out[tt * 128:(tt + 1) * 128, dc * 512:(dc + 1) * 512],
                            ob[:])

    nc.compile()
    return nc


_NC_CACHE = None


def _get_program():
    global _NC_CACHE
    if _NC_CACHE is None:
        _NC_CACHE = build_program()
    return _NC_CACHE


def prepare_inputs(x, q_w, kv_w, o_w, q_scale, k_scale, v_scale, segment_pos,
                   attn_mask):
    """Host-side prep: shard + transpose + fold scales + tables + masks."""
    x = np.asarray(x)
    q_w, kv_w, o_w = np.asarray(q_w), np.asarray(kv_w), np.asarray(o_w)
    q_scale, k_scale, v_scale = (np.asarray(q_scale), np.asarray(k_scale),
                                 np.asarray(v_scale))
    segment_pos = np.asarray(segment_pos)
    attn_mask = np.asarray(attn_mask)
    assert x.shape == (1, T, D)

    qs, ks, vs = 1.0 + q_scale, 1.0 + k_scale, 1.0 + v_scale
    qw_flat = (q_w * qs[None, None, :]).transpose(1, 0, 2).reshape(D, NH * H)
    kwk_flat = (kv_w[0] * ks[None, None, :]).transpose(1, 0, 2).reshape(D, KV * H)
    kwv_flat = (kv_w[1] * vs[None, None, :]).transpose(1, 0, 2).reshape(D, KV * H)
    ow_flat = o_w.reshape(NH * H, D)
    bf = ml_dtypes.bfloat16
    qw_b = np.ascontiguousarray(qw_flat, dtype=bf)
    kwk_b = np.ascontiguousarray(kwk_flat, dtype=bf)
    kwv_b = np.ascontiguousarray(kwv_flat, dtype=bf)
    ow_b = np.ascontiguousarray(ow_flat, dtype=bf)

    inv2q_arr = (qs ** -2.0).reshape(2, HH).T.astype(ml_dtypes.bfloat16)
    inv2k_arr = ((ks ** -2.0) / H).reshape(2, HH).T.astype(ml_dtypes.bfloat16)
    inv2v_arr = (np.tile(vs ** -2.0, KV) / H)[None, :].astype(np.float32)

    pos = segment_pos[0].astype(np.float64)
    freq = ROPE_BASE ** (2.0 * np.arange(HH) / H)
    xt_full = np.ascontiguousarray(x[0].T, dtype=bf)   # [D, T]
    am = attn_mask[0]                                  # [T, T] bool

    t_all = np.arange(T)
    in_maps = []
    for c in range(N_CORES):
        t_lo = c * TC
        xq_c = np.ascontiguousarray(xt_full[:, t_lo:t_lo + TC])

        ang = pos[t_lo:t_lo + TC][None, :] / freq[:, None]   # [HH, TC]
        cosq_c = np.cos(ang).astype(np.float32)
        sinq_c = np.sin(ang).astype(np.float32)

        s_idx = np.arange(t_lo - WINDOW, t_lo + TC)    # [SW]
        valid_s = s_idx >= 0
        sv = s_idx[valid_s]
        t_g = t_all[t_lo:t_lo + TC]
        m = np.zeros((SW, TC), dtype=bool)
        m[valid_s] = am[t_lo:t_lo + TC][:, sv].T
        dwin = t_g[None, :] - s_idx[:, None]
        m &= (dwin >= 0) & (dwin < WINDOW)
        maskT_c = np.where(m, 0.0, -4.0).astype(np.float32).reshape(NST, 128, TC)

        in_maps.append(dict(
            xq=xq_c, qw=qw_b, kwk=kwk_b, kwv=kwv_b, ow=ow_b,
            cosq=cosq_c, sinq=sinq_c, maskT=maskT_c,
            inv2q=inv2q_arr, inv2k=inv2k_arr, inv2v=inv2v_arr,
        ))
    return in_maps


def run(in_maps, trace=False, **kwargs):
    nc = _get_program()
    return run_bass_kernel_spmd(nc, in_maps, core_ids=list(range(N_CORES)),
                                trace=trace, **kwargs)


def kernel(**inputs) -> np.ndarray:
    in_maps = prepare_inputs(**inputs)
    res = run(in_maps)
    out = np.concatenate([res.results[c]["out"] for c in range(N_CORES)], axis=0)
    return out.reshape(1, T, D).astype(np.float32)


if __name__ == "__main__":
    nc = _get_program()
    print("built + compiled OK")


# revision 14
# speedup vs baseline: 1.1435x; 1.1435x over previous
"""Trainium2 Bass kernel for nn_Attention_28802050687686.

GQA sliding-window attention, T=4096, D=2048, 8 Q heads / 4 KV heads,
head_dim 256, window 1024, tanh soft-cap 50, RMSNorm+RoPE on Q/K, RMSNorm on V.

Sharding: sequence-parallel over 8 NeuronCores. Core c owns queries
[512c, 512c+512). Each core computes K/V for its OWN 512 rows only, then an
AllGather (via DRAM) distributes K/V; each core DMAs just its 1536-position
sliding window back into SBUF using partition-id-indexed dynamic offsets
(wrapped mod 8 — out-of-range chunks land in fully-masked positions).
"""
import sys

sys.path.insert(0, "/opt/trn_rl_repo")

import numpy as np
import ml_dtypes

import concourse.bass as bass
import concourse.tile as tile
from concourse import bacc, mybir
from concourse.bass_utils import run_bass_kernel_spmd

F32 = mybir.dt.float32
BF16 = mybir.dt.bfloat16
AF = mybir.ActivationFunctionType
OP = mybir.AluOpType

# problem constants
T, D, NH, KV, H, HH = 4096, 2048, 8, 4, 256, 128
N_CORES = 8
TC = 512          # queries / own kv rows per core
SW = 1536         # kv window per core
NST = SW // 128   # 12 s-tiles in window
NOT = TC // 128   # 4 own s-tiles
NDT = D // 128    # 16 d-tiles
NTT = TC // 128   # 4 t-tiles
WINDOW = 1024
SOFT_CAP = 50.0
EPS = 1e-6
ROPE_BASE = 10000.0

KCOLS = NH * TC            # 4096 cols of K in the kv-local pack (8 htiles x 512)
VCOLS = NOT * KV * 256     # 4096 cols of V pack
KVCOLS = KCOLS + VCOLS     # 8192


def build_program():
    nc = bacc.Bacc("TRN2", target_bir_lowering=False, debug=False)

    xq = nc.dram_tensor("xq", [D, TC], BF16, kind="ExternalInput").ap()
    qw = nc.dram_tensor("qw", [D, NH * H], BF16, kind="ExternalInput").ap()
    kwk = nc.dram_tensor("kwk", [D, KV * H], BF16, kind="ExternalInput").ap()
    kwv = nc.dram_tensor("kwv", [D, KV * H], BF16, kind="ExternalInput").ap()
    ow = nc.dram_tensor("ow", [NH * H, D], BF16, kind="ExternalInput").ap()
    cosq = nc.dram_tensor("cosq", [HH, TC], F32, kind="ExternalInput").ap()
    sinq = nc.dram_tensor("sinq", [HH, TC], F32, kind="ExternalInput").ap()
    maskT = nc.dram_tensor("maskT", [NST, 128, TC], F32, kind="ExternalInput").ap()
    inv2q = nc.dram_tensor("inv2q", [HH, 2], BF16, kind="ExternalInput").ap()
    inv2k = nc.dram_tensor("inv2k", [HH, 2], BF16, kind="ExternalInput").ap()
    inv2v = nc.dram_tensor("inv2v", [1, KV * H], F32, kind="ExternalInput").ap()
    out = nc.dram_tensor("out", [TC, D], F32, kind="ExternalOutput").ap()

    klocal = nc.dram_tensor("klocal", [128, KCOLS], BF16).ap()
    kgath = nc.dram_tensor("kgath", [N_CORES * 128, KCOLS], BF16,
                           addr_space="Shared").ap()
    vlocal = nc.dram_tensor("vlocal", [128, VCOLS], BF16).ap()
    vgath = nc.dram_tensor("vgath", [N_CORES * 128, VCOLS], BF16,
                           addr_space="Shared").ap()

    with tile.TileContext(nc) as tc:
        with tc.tile_pool(name="persist", bufs=1) as persist, \
             tc.tile_pool(name="work", bufs=2) as work, \
             tc.tile_pool(name="owp", bufs=2) as owp:
            kT_sb = persist.tile([128, KV * 2, SW], BF16)     # 24 KB/p
            V_sb = persist.tile([128, NST, KV, 256], BF16)    # 24 KB/p
            qT_sb = persist.tile([128, NH * 2, TC], BF16)     # 16 KB/p
            encT_sb = persist.tile([128, NH * 2, TC], BF16)   # 16 KB/p
            xq_ch = []
            for ch in range(4):
                xc = persist.tile([128, NDT // 4, TC], BF16, name=f"xq{ch}")
                nc.sync.dma_start(
                    xc[:], xq[ch * (D // 4):(ch + 1) * (D // 4), :].rearrange(
                        "(dt p) s -> p dt s", p=128))
                xq_ch.append(xc)

            def xq_sb(dt):
                return xq_ch[dt // 4][:, dt % 4, :]
            cosq_sb = persist.tile([HH, TC], F32)
            nc.sync.dma_start(cosq_sb[:], cosq[:])
            sinq_sb = persist.tile([HH, TC], F32)
            nc.sync.dma_start(sinq_sb[:], sinq[:])
            inv2q_sb = persist.tile([HH, 2], BF16)
            nc.sync.dma_start(inv2q_sb[:], inv2q[:])
            inv2k_sb = persist.tile([HH, 2], BF16)
            nc.sync.dma_start(inv2k_sb[:], inv2k[:])
            inv2v_sb = persist.tile([128, KV * H], F32)       # 4 KB/p
            nc.sync.dma_start(inv2v_sb[:], inv2v.to_broadcast([128, KV * H]))
            epsq1 = persist.tile([1, 1], F32)
            nc.vector.memset(epsq1[:], float(H) * EPS)
            epsk1 = persist.tile([1, 1], F32)
            nc.vector.memset(epsk1[:], EPS)
            eps128 = persist.tile([128, 1], F32)
            nc.vector.memset(eps128[:], EPS)
            ones_f = persist.tile([1, 128], BF16)
            nc.vector.memset(ones_f[:], 1.0)
            ones_b = persist.tile([128, 1], BF16)
            nc.vector.memset(ones_b[:], 1.0)

            def rope_norm_fold(ps_pair, inv2_sb, eps_t, dst0, dst1, bcast):
                """RMSNorm (exact via inv2 weights) + RoPE on an h-pair PSUM
                [128, 2, TC]; writes bf16 to dst0/dst1 [128, TC]."""
                sq0 = work.tile([128, TC], BF16, tag="wsq", name="sq0")
                nc.scalar.activation(sq0[:], ps_pair[:, 0, :], AF.Square)
                sq1 = work.tile([128, TC], BF16, tag="wsq", name="sq1")
                nc.scalar.activation(sq1[:], ps_pair[:, 1, :], AF.Square)
                rps = ps12.tile([1, TC], F32, tag="rowps", name="rps")
                nc.tensor.matmul(rps[:], inv2_sb[:, 0:1], sq0[:],
                                 start=True, stop=False)
                nc.tensor.matmul(rps[:], inv2_sb[:, 1:2], sq1[:],
                                 start=False, stop=True)
                srow = work.tile([1, TC], F32, tag="srow", name="srow")
                nc.scalar.activation(srow[:], rps[:], AF.Sqrt, bias=eps_t[:])
                rrow = work.tile([1, TC], F32, tag="rrow", name="rrow")
                nc.vector.reciprocal_approx_fast(rrow[:], srow[:])
                if bcast == "gpsimd":
                    rb = work.tile([128, TC], F32, tag="rb", name="rb")
                    nc.gpsimd.partition_broadcast(rb[:], rrow[:])
                else:
                    rrow_b = work.tile([1, TC], BF16, tag="rrowb", name="rrow_b")
                    nc.vector.tensor_copy(rrow_b[:], rrow[:])
                    rb = ps12.tile([128, TC], F32, tag="psv", name="rbps")
                    nc.tensor.matmul(rb[:], ones_f[:], rrow_b[:],
                                     start=True, stop=True)
                ta = work.tile([128, TC], F32, tag="wf", name="ta")
                nc.vector.tensor_tensor(ta[:], ps_pair[:, 0, :], cosq_sb[:], OP.mult)
                tb = work.tile([128, TC], F32, tag="wf", name="tb")
                nc.vector.tensor_tensor(tb[:], ps_pair[:, 1, :], sinq_sb[:], OP.mult)
                nc.vector.tensor_tensor(ta[:], ta[:], tb[:], OP.subtract)
                nc.vector.tensor_tensor(dst0, ta[:], rb[:], OP.mult)
                ta2 = work.tile([128, TC], F32, tag="wf", name="ta2")
                nc.vector.tensor_tensor(ta2[:], ps_pair[:, 1, :], cosq_sb[:], OP.mult)
                tb2 = work.tile([128, TC], F32, tag="wf", name="tb2")
                nc.vector.tensor_tensor(tb2[:], ps_pair[:, 0, :], sinq_sb[:], OP.mult)
                nc.vector.tensor_tensor(ta2[:], ta2[:], tb2[:], OP.add)
                nc.vector.tensor_tensor(dst1, ta2[:], rb[:], OP.mult)

            # ---------------- phase A: own-row K/V projections ----------------
            own0 = SW - TC  # own rows start at window col 1024
            with tc.tile_pool(name="wp", bufs=2) as wp, \
                 tc.tile_pool(name="ps12", bufs=2, space="PSUM") as ps12:
                pending = None
                for k in range(KV):
                    wk_sb = wp.tile([128, NDT, H], BF16, tag="wh", name="wk")
                    nc.sync.dma_start(
                        wk_sb[:],
                        kwk[:, k * H:(k + 1) * H].rearrange("(dt p) h -> p dt h", p=128))
                    psp = ps12.tile([128, 2, TC], F32, tag="pspair", name="pspK")
                    for hh in range(2):
                        for dt in range(NDT):
                            nc.tensor.matmul(
                                psp[:, hh, :],
                                wk_sb[:, dt, hh * 128:(hh + 1) * 128],
                                xq_sb(dt),
                                start=(dt == 0), stop=(dt == NDT - 1))
                    if pending is not None:
                        pp, pk = pending
                        rope_norm_fold(pp, inv2k_sb, epsk1,
                                       kT_sb[:, pk * 2 + 0, own0:SW],
                                       kT_sb[:, pk * 2 + 1, own0:SW], "gpsimd")
                    pending = (psp, k)
                pp, pk = pending
                rope_norm_fold(pp, inv2k_sb, epsk1,
                               kT_sb[:, pk * 2 + 0, own0:SW],
                               kT_sb[:, pk * 2 + 1, own0:SW], "gpsimd")

                nc.sync.dma_start(
                    klocal[:].rearrange("p (a b) -> p a b", a=NH),
                    kT_sb[:, :, own0:SW])
                nc.gpsimd.collective_compute(
                    "AllGather", OP.bypass,
                    replica_groups=[list(range(N_CORES))],
                    ins=[klocal[:]], outs=[kgath[:]],
                )

                def v_epilogue(psv, k, st):
                    sqv = work.tile([128, H], F32, tag="sqv", name="sqv")
                    nc.scalar.activation(sqv[:], psv[:], AF.Square)
                    sqw = work.tile([128, H], F32, tag="sqw", name="sqw")
                    nc.vector.tensor_tensor(
                        sqw[:], sqv[:], inv2v_sb[:, k * H:(k + 1) * H], OP.mult)
                    rv2 = work.tile([128, 1], F32, tag="rv2", name="rv2")
                    nc.vector.tensor_reduce(rv2[:], sqw[:],
                                            mybir.AxisListType.X, OP.add)
                    srv = work.tile([128, 1], F32, tag="srv", name="srv")
                    nc.scalar.activation(srv[:], rv2[:], AF.Sqrt, bias=eps128[:])
                    rv = work.tile([128, 1], F32, tag="rv", name="rv")
                    nc.vector.reciprocal_approx_fast(rv[:], srv[:])
                    nc.vector.tensor_scalar_mul(
                        V_sb[:, NST - NOT + st, k, :], psv[:], rv[:])

                pend_v = None
                for k in range(KV):
                    vw_sb = wp.tile([128, NDT, H], BF16, tag="wh", name="vw")
                    nc.sync.dma_start(
                        vw_sb[:],
                        kwv[:, k * H:(k + 1) * H].rearrange("(dt p) h -> p dt h", p=128))
                    for st in range(NOT):
                        psv = ps12.tile([128, H], F32, tag="psv", name="psv")
                        for dt in range(NDT):
                            nc.tensor.matmul(
                                psv[:],
                                xq_sb(dt)[:, st * 128:(st + 1) * 128],
                                vw_sb[:, dt, :],
                                start=(dt == 0), stop=(dt == NDT - 1))
                        if pend_v is not None:
                            v_epilogue(*pend_v)
                        pend_v = (psv, k, st)
                v_epilogue(*pend_v)

                nc.sync.dma_start(
                    vlocal[:].rearrange("p (a k c) -> p a k c", a=NOT, k=KV),
                    V_sb[:, NST - NOT:NST, :, :])
                nc.gpsimd.collective_compute(
                    "AllGather", OP.bypass,
                    replica_groups=[list(range(N_CORES))],
                    ins=[vlocal[:]], outs=[vgath[:]],
                )

                # ------------- phase B1: Q projections (overlap gather) -------
                pend_q = None
                for n in range(NH):
                    wq_sb = wp.tile([128, NDT, H], BF16, tag="wh", name="wq")
                    nc.sync.dma_start(
                        wq_sb[:],
                        qw[:, n * H:(n + 1) * H].rearrange("(dt p) h -> p dt h", p=128))
                    psp = ps12.tile([128, 2, TC], F32, tag="pspair", name="pspQ")
                    for hh in range(2):
                        for dt in range(NDT):
                            nc.tensor.matmul(
                                psp[:, hh, :],
                                wq_sb[:, dt, hh * 128:(hh + 1) * 128],
                                xq_sb(dt),
                                start=(dt == 0), stop=(dt == NDT - 1))
                    if pend_q is not None:
                        pp, pn = pend_q
                        rope_norm_fold(pp, inv2q_sb, epsq1,
                                       qT_sb[:, pn * 2 + 0, :],
                                       qT_sb[:, pn * 2 + 1, :], "pe")
                    pend_q = (psp, n)
                pp, pn = pend_q
                rope_norm_fold(pp, inv2q_sb, epsq1,
                               qT_sb[:, pn * 2 + 0, :], qT_sb[:, pn * 2 + 1, :],
                               "pe")

                # window chunks j=0,1 from the gathers (j=2 == own rows, already
                # in place). chunk index wraps mod 8: for cores 0/1 the wrapped
                # chunks are garbage but land in fully-masked window positions.
                # On gpsimd so the semaphore waits don't block the sync queue.
                pid = nc.gpsimd.partition_id()
                for j in range(2):
                    cj = ((pid + 6 + j) % N_CORES) * 128
                    nc.gpsimd.dma_start(
                        kT_sb[:, :, j * TC:(j + 1) * TC],
                        kgath[bass.ds(cj, 128), :].rearrange(
                            "p (a b) -> p a b", a=NH * 2))
                    nc.gpsimd.dma_start(
                        V_sb[:, NOT * j:NOT * (j + 1), :, :],
                        vgath[bass.ds(cj, 128), :].rearrange(
                            "p (a k c) -> p a k c", a=NOT, k=KV))

            # ---------------- phase B2: attention ----------------
            with tc.tile_pool(name="p3", bufs=1) as p3, \
                 tc.tile_pool(name="aw", bufs=4) as aw, \
                 tc.tile_pool(name="ps3", bufs=2, space="PSUM") as ps3, \
                 tc.tile_pool(name="psenc", bufs=2, space="PSUM") as psenc:
                maskT_sb = p3.tile([128, NST, TC], F32)       # 24 KB/p
                nc.sync.dma_start(maskT_sb[:], maskT.rearrange("j p t -> p j t"))

                for g in range(KV):
                    heads = (2 * g, 2 * g + 1)
                    encs = [psenc.tile([128, 2, TC], F32, tag="enc",
                                       name=f"enc{a}") for a in range(2)]
                    dens = [ps3.tile([1, TC], F32, tag="den", name=f"den{a}")
                            for a in range(2)]
                    def pv_step(pTs, st):
                        # enc.T[h, t] += V.T @ P.T ; den[t] += sum_s P.T
                        for hh in range(2):
                            for a in range(2):
                                nc.tensor.matmul(
                                    encs[a][:, hh, :],
                                    V_sb[:, st, g, hh * 128:(hh + 1) * 128],
                                    pTs[a][:],
                                    start=(st == 0), stop=(st == NST - 1))
                        for a in range(2):
                            nc.tensor.matmul(
                                dens[a][:], ones_b[:], pTs[a][:],
                                start=(st == 0), stop=(st == NST - 1))

                    pend_pv = None
                    for st in range(NST):
                        psLs = [ps3.tile([128, TC], F32, tag="psL",
                                         name=f"psL{a}") for a in range(2)]
                        for hh in range(2):
                            for a, n in enumerate(heads):
                                nc.tensor.matmul(
                                    psLs[a][:],
                                    kT_sb[:, g * 2 + hh, st * 128:(st + 1) * 128],
                                    qT_sb[:, n * 2 + hh, :],
                                    start=(hh == 0), stop=(hh == 1))
                        pTs = []
                        for a, n in enumerate(heads):
                            t1 = aw.tile([128, TC], F32, tag="t1", name="t1", bufs=3)
                            nc.scalar.activation(t1[:], psLs[a][:], AF.Tanh,
                                                 scale=1.0 / SOFT_CAP)
                            nc.vector.tensor_tensor(t1[:], t1[:],
                                                    maskT_sb[:, st, :], OP.add)
                            pT = aw.tile([128, TC], BF16, tag="pT", name="pT")
                            nc.scalar.activation(pT[:], t1[:], AF.Exp,
                                                 scale=SOFT_CAP)
                            pTs.append(pT)
                        if pend_pv is not None:
                            pv_step(*pend_pv)
                        pend_pv = (pTs, st)
                    pv_step(*pend_pv)
                    for a, n in enumerate(heads):
                        drow = aw.tile([1, TC], F32, tag="drow", name="drow", bufs=2)
                        nc.vector.reciprocal_approx_fast(drow[:], dens[a][:])
                        rbden = aw.tile([128, TC], F32, tag="rbden", name="rbden", bufs=2)
                        nc.gpsimd.partition_broadcast(rbden[:], drow[:])
                        for hh in range(2):
                            nc.vector.tensor_tensor(
                                encT_sb[:, n * 2 + hh, :], encs[a][:, hh, :],
                                rbden[:], OP.mult)

            # ---------------- phase C: output projection ----------------
            with tc.tile_pool(name="outp", bufs=3) as outp, \
                 tc.tile_pool(name="ps4", bufs=4, space="PSUM") as ps4:
                for dc in range(4):
                    ow_sb = owp.tile([128, NH * 2, 512], BF16, tag="ow", name="ow_sb")
                    nc.sync.dma_start(
                        ow_sb[:],
                        ow[:, dc * 512:(dc + 1) * 512].rearrange(
                            "(nh p) d -> p nh d", p=128))
                    for tt in range(NTT):
                        psO = ps4.tile([128, 512], F32, tag="psO", name="psO")
                        for nh in range(NH * 2):
                            nc.tensor.matmul(
                                psO[:],
                                encT_sb[:, nh, tt * 128:(tt + 1) * 128],
                                ow_sb[:, nh, :],
                                start=(nh == 0), stop=(nh == NH * 2 - 1))
                        ob = outp.tile([128, 512], F32, tag="ob", name="ob")
                        nc.vector.tensor_copy(ob[:], psO[:])
                        nc.sync.dma_start(
                            out[tt * 128:(tt + 1) * 128, dc * 512:(dc + 1) * 512],
                            ob[:])

    nc.compile()
    return nc


_NC_CACHE = None


def _get_program():
    global _NC_CACHE
    if _NC_CACHE is None:
        _NC_CACHE = build_program()
    return _NC_CACHE


def prepare_inputs(x, q_w, kv_w, o_w, q_scale, k_scale, v_scale, segment_pos,
                   attn_mask):
    """Host-side prep: shard + transpose + fold scales + tables + masks."""
    x = np.asarray(x)
    q_w, kv_w, o_w = np.asarray(q_w), np.asarray(kv_w), np.asarray(o_w)
    q_scale, k_scale, v_scale = (np.asarray(q_scale), np.asarray(k_scale),
                                 np.asarray(v_scale))
    segment_pos = np.asarray(segment_pos)
    attn_mask = np.asarray(attn_mask)
    assert x.shape == (1, T, D)

    qs, ks, vs = 1.0 + q_scale, 1.0 + k_scale, 1.0 + v_scale
    qw_flat = (q_w * qs[None, None, :]).transpose(1, 0, 2).reshape(D, NH * H)
    kwk_flat = (kv_w[0] * ks[None, None, :]).transpose(1, 0, 2).reshape(D, KV * H)
    kwv_flat = (kv_w[1] * vs[None, None, :]).transpose(1, 0, 2).reshape(D, KV * H)
    ow_flat = o_w.reshape(NH * H, D)
    bf = ml_dtypes.bfloat16
    qw_b = np.ascontiguousarray(qw_flat, dtype=bf)
    kwk_b = np.ascontiguousarray(kwk_flat, dtype=bf)
    kwv_b = np.ascontiguousarray(kwv_flat, dtype=bf)
    ow_b = np.ascontiguousarray(ow_flat, dtype=bf)

    inv2q_arr = (qs ** -2.0).reshape(2, HH).T.astype(ml_dtypes.bfloat16)
    inv2k_arr = ((ks ** -2.0) / H).reshape(2, HH).T.astype(ml_dtypes.bfloat16)
    inv2v_arr = (np.tile(vs ** -2.0, KV) / H)[None, :].astype(np.float32)

    pos = segment_pos[0].astype(np.float64)
    freq = ROPE_BASE ** (2.0 * np.arange(HH) / H)
    xt_full = np.ascontiguousarray(x[0].T, dtype=bf)   # [D, T]
    am = attn_mask[0]                                  # [T, T] bool

    t_all = np.arange(T)
    in_maps = []
    for c in range(N_CORES):
        t_lo = c * TC
        xq_c = np.ascontiguousarray(xt_full[:, t_lo:t_lo + TC])

        ang = pos[t_lo:t_lo + TC][None, :] / freq[:, None]   # [HH, TC]
        cosq_c = np.cos(ang).astype(np.float32)
        sinq_c = np.sin(ang).astype(np.float32)

        s_idx = np.arange(t_lo - WINDOW, t_lo + TC)    # [SW]
        valid_s = s_idx >= 0
        sv = s_idx[valid_s]
        t_g = t_all[t_lo:t_lo + TC]
        m = np.zeros((SW, TC), dtype=bool)
        m[valid_s] = am[t_lo:t_lo + TC][:, sv].T
        dwin = t_g[None, :] - s_idx[:, None]
        m &= (dwin >= 0) & (dwin < WINDOW)
        maskT_c = np.where(m, 0.0, -4.0).astype(np.float32).reshape(NST, 128, TC)

        in_maps.append(dict(
            xq=xq_c, qw=qw_b, kwk=kwk_b, kwv=kwv_b, ow=ow_b,
            cosq=cosq_c, sinq=sinq_c, maskT=maskT_c,
            inv2q=inv2q_arr, inv2k=inv2k_arr, inv2v=inv2v_arr,
        ))
    return in_maps


def run(in_maps, trace=False, **kwargs):
    nc = _get_program()
    return run_bass_kernel_spmd(nc, in_maps, core_ids=list(range(N_CORES)),
                                trace=trace, **kwargs)


def kernel(**inputs) -> np.ndarray:
    in_maps = prepare_inputs(**inputs)
    res = run(in_maps)
    out = np.concatenate([res.results[c]["out"] for c in range(N_CORES)], axis=0)
    return out.reshape(1, T, D).astype(np.float32)


if __name__ == "__main__":
    nc = _get_program()
    print("built + compiled OK")
